# revision 10
# baseline (speedup 1.0000x reference)
"""Trainium2 Bass kernel for nn_Decoder (2-layer GRU decoder, weight-tied vocab projection).

Strategy (8 NeuronCores, SPMD):
  - Tensor-parallel recurrence: each core computes a 128-row slice of every GRU
    gate (H=1024 -> 8 x 128). Per superstep, one fused AllGather exchanges the
    new h0/h1 slices (bf16) across cores.
  - Gate preactivations are built entirely in PSUM by fused matmul groups
    (w_hh @ h  +  w_ih @ x  + bias-outer-product), fp32 accumulate.
  - The h-state for the "z*h_prev" path is kept in fp32 locally (h_own); only
    matmul operands are bf16.
  - Vocab-tied projection: embedding^T is sharded 4000 cols/core; the MLP is
    computed replicated (cheap) and logits are vocab-sharded.
Output: (B=16, S=128, V=32000) fp32, assembled host-side.
"""
import os
import numpy as np
import ml_dtypes

V, E, H, L = 32000, 512, 1024, 2
B, S = 16, 128
N = 8                  # cores
HS = H // N            # 128 rows of h per core
VS = V // N            # 4000 vocab cols per core
TB = B * S             # 2048 (t,b) rows
STEPS = int(os.environ.get("K_STEPS", str(S)))  # reduced for smoke testing

_cache = {}


def _build():
    import concourse.bass as bass
    import concourse.bacc as bacc
    import concourse.mybir as mybir
    import concourse.tile as tile
    from concourse.masks import make_identity

    fp32 = mybir.dt.float32
    bf16 = mybir.dt.bfloat16
    i32 = mybir.dt.int32
    T = STEPS
    NT = TB // 128 if T == S else (T * B) // 128   # number of 128-row (t,b) tiles
    RW = 64            # h0 ring: 4 slots x 16 cols per k-tile

    nc = bacc.Bacc("TRN2", num_devices=N, target_bir_lowering=False)

    # ---- DRAM I/O ----
    emb = nc.dram_tensor("emb", [V, E], fp32, kind="ExternalInput")
    idx = nc.dram_tensor("idx", [B * T // 128 * 8 if False else (T * B // 128), 128], i32, kind="ExternalInput")
    h0f = nc.dram_tensor("h0f", [H, B], bf16, kind="ExternalInput")
    h1f = nc.dram_tensor("h1f", [H, B], bf16, kind="ExternalInput")
    h0o = nc.dram_tensor("h0o", [HS, B], fp32, kind="ExternalInput")
    h1o = nc.dram_tensor("h1o", [HS, B], fp32, kind="ExternalInput")
    wih0 = nc.dram_tensor("wih0", [E, 3 * HS], bf16, kind="ExternalInput")
    whh0 = nc.dram_tensor("whh0", [H, 3 * HS], bf16, kind="ExternalInput")
    wih1 = nc.dram_tensor("wih1", [H, 3 * HS], bf16, kind="ExternalInput")
    whh1 = nc.dram_tensor("whh1", [H, 3 * HS], bf16, kind="ExternalInput")
    ba0 = nc.dram_tensor("ba0", [1, 3 * HS], bf16, kind="ExternalInput")
    bb0 = nc.dram_tensor("bb0", [1, HS], bf16, kind="ExternalInput")
    ba1 = nc.dram_tensor("ba1", [1, 3 * HS], bf16, kind="ExternalInput")
    bb1 = nc.dram_tensor("bb1", [1, HS], bf16, kind="ExternalInput")
    w1t = nc.dram_tensor("w1t", [H, H], bf16, kind="ExternalInput")
    b1c = nc.dram_tensor("b1c", [128, 8], fp32, kind="ExternalInput")
    w2t = nc.dram_tensor("w2t", [H, E], bf16, kind="ExternalInput")
    b2c = nc.dram_tensor("b2c", [128, 4], fp32, kind="ExternalInput")
    embts = nc.dram_tensor("embts", [E, VS], bf16, kind="ExternalInput")
    bgen = nc.dram_tensor("bgen", [1, VS], bf16, kind="ExternalInput")
    out = nc.dram_tensor("out", [T * B, VS], fp32, kind="ExternalOutput")

    with tile.TileContext(nc) as tc:
        with (
            tc.tile_pool(name="wp", bufs=1) as wp,
            tc.tile_pool(name="state", bufs=1) as st,
            tc.tile_pool(name="dram", bufs=4, space="DRAM") as dr,
        ):
            # ---- resident weights ----
            def load3d(name, src, kdim, cols):
                # src (kdim*128, cols) -> sbuf (128, kdim*cols), k-major blocks
                tl = wp.tile([128, kdim * cols], bf16, tag=name, name=name)
                nc.sync.dma_start(
                    tl[:].rearrange("p (k c) -> p k c", k=kdim),
                    src[:].rearrange("(k p) c -> p k c", p=128))
                return tl

            wih0_sb = load3d("wih0_sb", wih0, 4, 3 * HS)
            whh0_sb = load3d("whh0_sb", whh0, 8, 3 * HS)
            wih1_sb = load3d("wih1_sb", wih1, 8, 3 * HS)
            whh1_sb = load3d("whh1_sb", whh1, 8, 3 * HS)
            w1t_sb = load3d("w1t_sb", w1t, 8, H)
            w2t_sb = load3d("w2t_sb", w2t, 8, E)
            embts_sb = load3d("embts_sb", embts, 4, VS)
            ba0_sb = wp.tile([128, 3 * HS], bf16, tag="ba0s", name="ba0s")
            nc.sync.dma_start(ba0_sb[0:1, :], ba0[:])
            bb0_sb = wp.tile([128, HS], bf16, tag="bb0s", name="bb0s")
            nc.sync.dma_start(bb0_sb[0:1, :], bb0[:])
            ba1_sb = wp.tile([128, 3 * HS], bf16, tag="ba1s", name="ba1s")
            nc.sync.dma_start(ba1_sb[0:1, :], ba1[:])
            bb1_sb = wp.tile([128, HS], bf16, tag="bb1s", name="bb1s")
            nc.sync.dma_start(bb1_sb[0:1, :], bb1[:])
            b1_sb = wp.tile([128, 8], fp32, tag="b1s", name="b1s")
            nc.sync.dma_start(b1_sb[:], b1c[:])
            b2_sb = wp.tile([128, 4], fp32, tag="b2s", name="b2s")
            nc.sync.dma_start(b2_sb[:], b2c[:])
            bgen_sb = wp.tile([128, VS], bf16, tag="bgens", name="bgens")
            nc.sync.dma_start(bgen_sb[0:1, :], bgen[:])
            ones_sb = wp.tile([128, 128], bf16, tag="ones", name="ones")
            nc.vector.memset(ones_sb[:], 1.0)

            # ---- state ----
            XT = st.tile([128, 4 * (T * B)], bf16, tag="XT", name="XT")        # x^T, 4 E-tiles
            ring0 = st.tile([128, 4 * 128], bf16, tag="ring0", name="ring0")   # h0 ring, 4 slots x (8k x 16)
            H1T = st.tile([128, (T + 1) * 128], bf16, tag="H1T", name="H1T")   # slot-major
            h_own0 = [st.tile([128, B], fp32, tag=f"ho0_{p}", name=f"ho0_{p}") for p in range(2)]
            h_own1 = [st.tile([128, B], fp32, tag=f"ho1_{p}", name=f"ho1_{p}") for p in range(2)]
            HW1 = 16 * (T + 1)

            # init h state
            nc.sync.dma_start(
                ring0[:, 0:128].rearrange("p (k c) -> p k c", k=8),
                h0f[:].rearrange("(k p) c -> p k c", p=128))
            nc.sync.dma_start(
                H1T[:, 0:128].rearrange("p (k c) -> p k c", k=8),
                h1f[:].rearrange("(k p) c -> p k c", p=128))
            nc.sync.dma_start(h_own0[0][:], h0o[:])
            nc.sync.dma_start(h_own1[0][:], h1o[:])

            # ---- P1: gather + transpose x ----
            with (
                tc.tile_pool(name="gp", bufs=2) as gp,
                tc.tile_pool(name="gps", bufs=2, space="PSUM") as gpp,
            ):
                idx_sb = gp.tile([128, NT], i32, tag="idx", name="idx")
                nc.sync.dma_start(idx_sb[:], idx[:].rearrange("a b -> b a"))
                ident = gp.tile([128, 128], fp32, tag="ident", name="ident")
                make_identity(nc, ident[:])
                for i in range(NT):
                    xg = gp.tile([128, E], fp32, tag="xg", name=f"xg{i}", bufs=2)
                    nc.gpsimd.indirect_dma_start(
                        out=xg[:], out_offset=None, in_=emb[:],
                        in_offset=bass.IndirectOffsetOnAxis(ap=idx_sb[:, i:i + 1], axis=0))
                    for e in range(4):
                        pt = gpp.tile([128, 128], fp32, tag="pt", name=f"pt{i}_{e}")
                        nc.tensor.transpose(pt[:], xg[:, 128 * e:128 * (e + 1)], ident[:])
                        nc.vector.tensor_copy(
                            XT[:, (T * B) * e + 128 * i: (T * B) * e + 128 * (i + 1)], pt[:])

            # ---- P2: recurrence supersteps ----
            def gate_groups(ps, whh_sb, wih_sb, kdim_i, rhs_h_fn, rhs_x_fn, baS, bbS):
                # build r,z,i_n,h_n psum groups into ps(128,64)
                for gi, (c0, gc) in enumerate([(0, 0), (16, HS), (32, 2 * HS)]):
                    # r (c0=0), z (c0=16): h-part + x-part + bias; i_n (c0=32): x-part + bias
                    mms = []
                    if gi < 2:
                        for k in range(8):
                            mms.append((whh_sb[:, 3 * HS * k + gc:3 * HS * k + gc + 128], rhs_h_fn(k)))
                    for k in range(kdim_i):
                        mms.append((wih_sb[:, 3 * HS * k + gc:3 * HS * k + gc + 128], rhs_x_fn(k)))
                    mms.append((baS[0:1, gc:gc + 128], ones_sb[0:1, 0:B]))
                    for mi, (lt, rr) in enumerate(mms):
                        nc.tensor.matmul(ps[:, c0:c0 + 16], lt, rr,
                                         start=(mi == 0), stop=(mi == len(mms) - 1))
                # h_n group (cols 48:64)
                mms = [(whh_sb[:, 3 * HS * k + 2 * HS:3 * HS * k + 2 * HS + 128], rhs_h_fn(k))
                       for k in range(8)]
                mms.append((bbS[0:1, 0:HS], ones_sb[0:1, 0:B]))
                for mi, (lt, rr) in enumerate(mms):
                    nc.tensor.matmul(ps[:, 48:64], lt, rr,
                                     start=(mi == 0), stop=(mi == len(mms) - 1))

            def gates(ps, hprev, hnew, cc_dst, tag, t):
                rz = st.tile([128, 32], fp32, tag=f"rz{tag}", name=f"rz{tag}_{t}", bufs=2)
                nc.scalar.activation(rz[:], ps[:, 0:32], mybir.ActivationFunctionType.Sigmoid)
                t1 = st.tile([128, B], fp32, tag=f"t1{tag}", name=f"t1{tag}_{t}", bufs=2)
                nc.vector.tensor_tensor(out=t1[:], in0=ps[:, 48:64], in1=rz[:, 0:16],
                                        op=mybir.AluOpType.mult)
                nc.vector.tensor_tensor(out=t1[:], in0=t1[:], in1=ps[:, 32:48],
                                        op=mybir.AluOpType.add)
                nsb = st.tile([128, B], fp32, tag=f"n{tag}", name=f"n{tag}_{t}", bufs=2)
                nc.scalar.activation(nsb[:], t1[:], mybir.ActivationFunctionType.Tanh)
                a = st.tile([128, B], fp32, tag=f"a{tag}", name=f"a{tag}_{t}", bufs=2)
                nc.vector.tensor_tensor(out=a[:], in0=hprev[:], in1=nsb[:],
                                        op=mybir.AluOpType.subtract)
                nc.vector.tensor_tensor(out=a[:], in0=a[:], in1=rz[:, 16:32],
                                        op=mybir.AluOpType.mult)
                nc.vector.tensor_tensor(out=hnew[:], in0=a[:], in1=nsb[:],
                                        op=mybir.AluOpType.add)
                nc.vector.tensor_copy(cc_dst, hnew[:])

            psp_cm = tc.tile_pool(name="ps", bufs=2, space="PSUM")
            psp = psp_cm.__enter__()
            mp_cm = tc.tile_pool(name="mp", bufs=1)
            mp = mp_cm.__enter__()
            mpp_cm = tc.tile_pool(name="mps", bufs=1, space="PSUM")
            mpp = mpp_cm.__enter__()

            cc_srcs = {t: st.tile([128, 32], bf16, tag="cc_src", name=f"cc_src{t}", bufs=3)
                       for t in range(1, T + 3)}
            nc.vector.memset(cc_srcs[1][:, 16:32], 0.0)
            nc.vector.memset(cc_srcs[2][:, 16:32], 0.0)
            nc.vector.memset(cc_srcs[T + 1][:, 0:16], 0.0)
            nc.vector.memset(cc_srcs[T + 2][:, 0:16], 0.0)

            GC = 128
            NG = (T * B) // GC
            H1T3 = H1T[:].rearrange("p (t c) -> p t c", c=128)

            p5_state = {}

            def p5_items(g):
                # returns list of emission closures for tb-row tile g (128 rows)
                items = []

                def rt_item(m, g=g):
                    if ("RT", g) not in p5_state:
                        p5_state[("RT", g)] = mp.tile([128, 8 * GC], bf16, tag="RTg",
                                                      name=f"RTg{g}", bufs=2)
                    RTg = p5_state[("RT", g)]
                    pr = mpp.tile([128, GC], fp32, tag="pr", name=f"pr{g}_{m}", bufs=1)
                    for k in range(8):
                        nc.tensor.matmul(
                            pr[:], w1t_sb[:, H * k + 128 * m:H * k + 128 * (m + 1)],
                            H1T3[:, 8 * g + 1:8 * (g + 1) + 1, 16 * k:16 * (k + 1)],
                            start=(k == 0), stop=(k == 7))
                    nc.scalar.activation(
                        RTg[:, GC * m:GC * (m + 1)], pr[:],
                        mybir.ActivationFunctionType.Relu, bias=b1_sb[:, m:m + 1], scale=1.0)

                def out_item(m, g=g):
                    if ("OUT", g) not in p5_state:
                        p5_state[("OUT", g)] = mp.tile([128, 4 * GC], bf16, tag="OUTg",
                                                       name=f"OUTg{g}", bufs=2)
                    RTg = p5_state[("RT", g)]
                    OUTg = p5_state[("OUT", g)]
                    po = mpp.tile([128, GC], fp32, tag="po", name=f"po{g}_{m}", bufs=1)
                    for k in range(8):
                        nc.tensor.matmul(
                            po[:], w2t_sb[:, E * k + 128 * m:E * k + 128 * (m + 1)],
                            RTg[:, GC * k:GC * (k + 1)],
                            start=(k == 0), stop=(k == 7))
                    nc.scalar.activation(
                        OUTg[:, GC * m:GC * (m + 1)], po[:],
                        mybir.ActivationFunctionType.Identity, bias=b2_sb[:, m:m + 1], scale=1.0)

                def lg_item(nchs, g=g):
                    if ("L", g) not in p5_state:
                        p5_state[("L", g)] = mp.tile([128, VS], fp32, tag="lsb",
                                                     name=f"lsb{g}", bufs=2)
                    OUTg = p5_state[("OUT", g)]
                    lsb = p5_state[("L", g)]
                    pl = mpp.tile([128, 500], fp32, tag="pl", name=f"pl{g}_{nchs}", bufs=2)
                    for e in range(4):
                        nc.tensor.matmul(
                            pl[:], OUTg[:, GC * e:GC * e + 128],
                            embts_sb[:, VS * e + nchs:VS * e + nchs + 500],
                            start=(e == 0), stop=False)
                    nc.tensor.matmul(
                        pl[:], ones_sb[0:1, :], bgen_sb[0:1, nchs:nchs + 500],
                        start=False, stop=True)
                    nc.vector.tensor_copy(lsb[:, nchs:nchs + 500], pl[:])

                def dma_item(g=g):
                    lsb = p5_state[("L", g)]
                    nc.gpsimd.dma_start(out[128 * g:128 * (g + 1), 0:VS // 2],
                                        lsb[:, 0:VS // 2])
                    nc.gpsimd.dma_start(out[128 * g:128 * (g + 1), VS // 2:VS],
                                        lsb[:, VS // 2:VS])

                for m in range(8):
                    items.append(lambda m=m: rt_item(m))
                for m in range(4):
                    items.append(lambda m=m: out_item(m))
                for nchs in range(0, VS, 500):
                    items.append(lambda n=nchs: lg_item(n))
                items.append(dma_item)
                return items

            p5_queue = []
            p5_next_g = 0

            def p5_drain(t, budget):
                nonlocal_ns = {}
                return budget

            for t in range(1, T + 3):
                cc_src = cc_srcs[t]
                if t <= T:
                    # layer 0: h0_t  (critical chain)
                    ps0 = psp.tile([128, 64], fp32, tag="ps0", name=f"ps0_{t}", bufs=2)
                    so = 16 * ((t - 1) % 4)
                    gate_groups(
                        ps0, whh0_sb, wih0_sb, 4,
                        lambda k: ring0[:, 128 * ((t - 1) % 4) + 16 * k:128 * ((t - 1) % 4) + 16 * k + 16],
                        lambda e: XT[:, (T * B) * e + B * (t - 1):(T * B) * e + B * t],
                        ba0_sb, bb0_sb)
                    gates(ps0, h_own0[(t - 1) % 2], h_own0[t % 2], cc_src[:, 0:16], "L0", t)

                ccin = dr.tile([128, 32], bf16, tag="ccin", name=f"ccin{t}", bufs=2)
                ccout = dr.tile([128 * N, 32], bf16, tag="ccout", name=f"ccout{t}",
                                bufs=2, addr_space="Shared")
                nc.sync.dma_start(ccin[:], cc_src[:])
                nc.gpsimd.collective_compute(
                    "AllGather", mybir.AluOpType.bypass,
                    replica_groups=[list(range(N))],
                    ins=[ccin[:]], outs=[ccout[:]])
                if t <= T:
                    nc.sync.dma_start(
                        ring0[:, 128 * (t % 4):128 * (t % 4) + 128].rearrange("p (k c) -> p k c", k=8),
                        ccout[:, 0:16].rearrange("(k p) c -> p k c", p=128))
                if t >= 3:
                    nc.sync.dma_start(
                        H1T[:, 128 * (t - 2):128 * (t - 1)].rearrange("p (k c) -> p k c", k=8),
                        ccout[:, 16:32].rearrange("(k p) c -> p k c", p=128))

                if 2 <= t <= T + 1:
                    # layer 1: h1_{t-1}, off the critical chain (rides AG t+1)
                    ps1 = psp.tile([128, 64], fp32, tag="ps1", name=f"ps1_{t}", bufs=2)
                    so1 = 16 * ((t - 1) % 4)
                    gate_groups(
                        ps1, whh1_sb, wih1_sb, 8,
                        lambda k: H1T[:, 128 * (t - 2) + 16 * k:128 * (t - 2) + 16 * k + 16],
                        lambda k: ring0[:, 128 * ((t - 1) % 4) + 16 * k:128 * ((t - 1) % 4) + 16 * k + 16],
                        ba1_sb, bb1_sb)
                    gates(ps1, h_own1[(t - 2) % 2], h_own1[(t - 1) % 2],
                          cc_srcs[t + 1][:, 16:32], "L1", t)

                # interleave vocab-projection work once its h1 block landed
                while p5_next_g < NG and t >= 8 * p5_next_g + 10:
                    p5_queue.extend(p5_items(p5_next_g))
                    p5_next_g += 1
                for _ in range(3):
                    if p5_queue:
                        p5_queue.pop(0)()
            while p5_next_g < NG:
                p5_queue.extend(p5_items(p5_next_g))
                p5_next_g += 1
            for it in p5_queue:
                it()
            p5_queue = []

            mpp_cm.__exit__(None, None, None)
            mp_cm.__exit__(None, None, None)
            psp_cm.__exit__(None, None, None)

    nc.finalize()
    return nc


def _prep_inputs(hidden, trg, embedding, w_ih0, w_hh0, b_ih0, b_hh0,
                 w_ih1, w_hh1, b_ih1, b_hh1, w1, b1, w2, b2, b_gen):
    bf = ml_dtypes.bfloat16
    T = STEPS
    f32 = np.float32
    hidden = np.asarray(hidden, f32)
    trg = np.asarray(trg)
    embedding = np.asarray(embedding, f32)
    in_maps = []
    # (t,b) index order
    idx_full = np.asarray(trg.T[:T], np.int32).reshape(-1)          # (T*B,)
    idx_tiles = idx_full.reshape(-1, 128).astype(np.int32)          # (NT,128)

    def gslice(wT, r):
        # wT (K, 3H) -> (K, 384) slice of each gate for core r
        cols = np.concatenate([np.arange(HS) + g * H + r * HS for g in range(3)])
        return np.ascontiguousarray(wT[:, cols])

    for r in range(N):
        sl = slice(r * HS, (r + 1) * HS)
        ba0_ = (b_ih0 + b_hh0).astype(f32)
        ba0v = np.concatenate([ba0_[0 * H + r * HS:0 * H + (r + 1) * HS],
                               ba0_[1 * H + r * HS:1 * H + (r + 1) * HS],
                               np.asarray(b_ih0, f32)[2 * H + r * HS:2 * H + (r + 1) * HS]])
        bb0v = np.asarray(b_hh0, f32)[2 * H + r * HS:2 * H + (r + 1) * HS]
        ba1_ = (b_ih1 + b_hh1).astype(f32)
        ba1v = np.concatenate([ba1_[0 * H + r * HS:0 * H + (r + 1) * HS],
                               ba1_[1 * H + r * HS:1 * H + (r + 1) * HS],
                               np.asarray(b_ih1, f32)[2 * H + r * HS:2 * H + (r + 1) * HS]])
        bb1v = np.asarray(b_hh1, f32)[2 * H + r * HS:2 * H + (r + 1) * HS]
        in_maps.append({
            "emb": embedding,
            "idx": idx_tiles,
            "h0f": hidden[0].T.astype(bf),
            "h1f": hidden[1].T.astype(bf),
            "h0o": np.ascontiguousarray(hidden[0].T[sl]).astype(f32),
            "h1o": np.ascontiguousarray(hidden[1].T[sl]).astype(f32),
            "wih0": gslice(np.asarray(w_ih0, f32).T, r).astype(bf),
            "whh0": gslice(np.asarray(w_hh0, f32).T, r).astype(bf),
            "wih1": gslice(np.asarray(w_ih1, f32).T, r).astype(bf),
            "whh1": gslice(np.asarray(w_hh1, f32).T, r).astype(bf),
            "ba0": ba0v.reshape(1, -1).astype(bf),
            "bb0": bb0v.reshape(1, -1).astype(bf),
            "ba1": ba1v.reshape(1, -1).astype(bf),
            "bb1": bb1v.reshape(1, -1).astype(bf),
            "w1t": np.asarray(w1, f32).T.astype(bf),
            "b1c": np.asarray(b1, f32).reshape(8, 128).T.astype(f32),
            "w2t": np.asarray(w2, f32).T.astype(bf),
            "b2c": np.asarray(b2, f32).reshape(4, 128).T.astype(f32),
            "embts": np.ascontiguousarray(embedding.T[:, r * VS:(r + 1) * VS]).astype(bf),
            "bgen": np.asarray(b_gen, f32)[r * VS:(r + 1) * VS].reshape(1, -1).astype(bf),
        })
    return in_maps


def kernel(**inputs):
    from concourse.bass_utils import run_bass_kernel_spmd
    if "nc" not in _cache:
        _cache["nc"] = _build()
    nc = _cache["nc"]
    in_maps = _prep_inputs(**inputs)
    res = run_bass_kernel_spmd(nc, in_maps, core_ids=list(range(N)))
    T = STEPS
    outf = np.empty((B, T, V), np.float32)
    for r in range(N):
        lr = res.results[r]["out"].reshape(T, B, VS)
        outf[:, :, r * VS:(r + 1) * VS] = lr.transpose(1, 0, 2)
    return outf


# revision 15
# speedup vs baseline: 1.0201x; 1.0201x over previous
"""Trainium2 Bass kernel for nn_Decoder (2-layer GRU decoder, weight-tied vocab projection).

Strategy (8 NeuronCores, SPMD):
  - Tensor-parallel recurrence: each core computes a 128-row slice of every GRU
    gate (H=1024 -> 8 x 128). Per superstep, one fused AllGather exchanges the
    new h0/h1 slices (bf16) across cores.
  - Gate preactivations are built entirely in PSUM by fused matmul groups
    (w_hh @ h  +  w_ih @ x  + bias-outer-product), fp32 accumulate.
  - The h-state for the "z*h_prev" path is kept in fp32 locally (h_own); only
    matmul operands are bf16.
  - Vocab-tied projection: embedding^T is sharded 4000 cols/core; the MLP is
    computed replicated (cheap) and logits are vocab-sharded.
Output: (B=16, S=128, V=32000) fp32, assembled host-side.
"""
import os
import numpy as np
import ml_dtypes

V, E, H, L = 32000, 512, 1024, 2
B, S = 16, 128
N = 8                  # cores
HS = H // N            # 128 rows of h per core
VS = V // N            # 4000 vocab cols per core
TB = B * S             # 2048 (t,b) rows
STEPS = int(os.environ.get("K_STEPS", str(S)))  # reduced for smoke testing

_cache = {}


def _build(USE_BIAS=True):
    import concourse.bass as bass
    import concourse.bacc as bacc
    import concourse.mybir as mybir
    import concourse.tile as tile
    from concourse.masks import make_identity

    fp32 = mybir.dt.float32
    bf16 = mybir.dt.bfloat16
    i32 = mybir.dt.int32
    T = STEPS
    NT = TB // 128 if T == S else (T * B) // 128   # number of 128-row (t,b) tiles
    RW = 64            # h0 ring: 4 slots x 16 cols per k-tile

    nc = bacc.Bacc("TRN2", num_devices=N, target_bir_lowering=False)

    # ---- DRAM I/O ----
    emb = nc.dram_tensor("emb", [V, E], fp32, kind="ExternalInput")
    idx = nc.dram_tensor("idx", [B * T // 128 * 8 if False else (T * B // 128), 128], i32, kind="ExternalInput")
    h0f = nc.dram_tensor("h0f", [H, B], bf16, kind="ExternalInput")
    h1f = nc.dram_tensor("h1f", [H, B], bf16, kind="ExternalInput")
    h0o = nc.dram_tensor("h0o", [HS, B], fp32, kind="ExternalInput")
    h1o = nc.dram_tensor("h1o", [HS, B], fp32, kind="ExternalInput")
    wih0 = nc.dram_tensor("wih0", [E, 3 * HS], bf16, kind="ExternalInput")
    whh0 = nc.dram_tensor("whh0", [H, 3 * HS], bf16, kind="ExternalInput")
    wih1 = nc.dram_tensor("wih1", [H, 3 * HS], bf16, kind="ExternalInput")
    whh1 = nc.dram_tensor("whh1", [H, 3 * HS], bf16, kind="ExternalInput")
    ba0 = nc.dram_tensor("ba0", [1, 3 * HS], bf16, kind="ExternalInput")
    bb0 = nc.dram_tensor("bb0", [1, HS], bf16, kind="ExternalInput")
    ba1 = nc.dram_tensor("ba1", [1, 3 * HS], bf16, kind="ExternalInput")
    bb1 = nc.dram_tensor("bb1", [1, HS], bf16, kind="ExternalInput")
    w1t = nc.dram_tensor("w1t", [H, H], bf16, kind="ExternalInput")
    b1c = nc.dram_tensor("b1c", [128, 8], fp32, kind="ExternalInput")
    w2t = nc.dram_tensor("w2t", [H, E], bf16, kind="ExternalInput")
    b2c = nc.dram_tensor("b2c", [128, 4], fp32, kind="ExternalInput")
    embts = nc.dram_tensor("embts", [E, VS], bf16, kind="ExternalInput")
    bgen = nc.dram_tensor("bgen", [1, VS], bf16, kind="ExternalInput")
    out = nc.dram_tensor("out", [T * B, VS], fp32, kind="ExternalOutput")

    with tile.TileContext(nc) as tc:
        with (
            tc.tile_pool(name="wp", bufs=1) as wp,
            tc.tile_pool(name="state", bufs=1) as st,
            tc.tile_pool(name="dram", bufs=4, space="DRAM") as dr,
        ):
            # ---- resident weights ----
            def load3d(name, src, kdim, cols):
                # src (kdim*128, cols) -> sbuf (128, kdim*cols), k-major blocks
                tl = wp.tile([128, kdim * cols], bf16, tag=name, name=name)
                nc.sync.dma_start(
                    tl[:].rearrange("p (k c) -> p k c", k=kdim),
                    src[:].rearrange("(k p) c -> p k c", p=128))
                return tl

            wih0_sb = load3d("wih0_sb", wih0, 4, 3 * HS)
            whh0_sb = load3d("whh0_sb", whh0, 8, 3 * HS)
            wih1_sb = load3d("wih1_sb", wih1, 8, 3 * HS)
            whh1_sb = load3d("whh1_sb", whh1, 8, 3 * HS)
            w1t_sb = load3d("w1t_sb", w1t, 8, H)
            w2t_sb = load3d("w2t_sb", w2t, 8, E)
            embts_sb = load3d("embts_sb", embts, 4, VS)
            ba0_sb = wp.tile([128, 3 * HS], bf16, tag="ba0s", name="ba0s")
            nc.sync.dma_start(ba0_sb[0:1, :], ba0[:])
            bb0_sb = wp.tile([128, HS], bf16, tag="bb0s", name="bb0s")
            nc.sync.dma_start(bb0_sb[0:1, :], bb0[:])
            ba1_sb = wp.tile([128, 3 * HS], bf16, tag="ba1s", name="ba1s")
            nc.sync.dma_start(ba1_sb[0:1, :], ba1[:])
            bb1_sb = wp.tile([128, HS], bf16, tag="bb1s", name="bb1s")
            nc.sync.dma_start(bb1_sb[0:1, :], bb1[:])
            b1_sb = wp.tile([128, 8], fp32, tag="b1s", name="b1s")
            nc.sync.dma_start(b1_sb[:], b1c[:])
            b2_sb = wp.tile([128, 4], fp32, tag="b2s", name="b2s")
            nc.sync.dma_start(b2_sb[:], b2c[:])
            bgen_sb = wp.tile([128, VS], bf16, tag="bgens", name="bgens")
            nc.sync.dma_start(bgen_sb[0:1, :], bgen[:])
            ones_sb = wp.tile([128, 512], bf16, tag="ones", name="ones")
            nc.vector.memset(ones_sb[:], 1.0)

            # ---- state ----
            XT = st.tile([128, 4 * (T * B)], bf16, tag="XT", name="XT")        # x^T, 4 E-tiles
            ring0 = st.tile([128, 4 * 128], bf16, tag="ring0", name="ring0")   # h0 ring, 4 slots x (8k x 16)
            H1T = st.tile([128, (T + 1) * 128], bf16, tag="H1T", name="H1T")   # slot-major
            h_own0 = [st.tile([128, B], fp32, tag=f"ho0_{p}", name=f"ho0_{p}") for p in range(2)]
            h_own1 = [st.tile([128, B], fp32, tag=f"ho1_{p}", name=f"ho1_{p}") for p in range(2)]
            HW1 = 16 * (T + 1)

            # init h state
            nc.sync.dma_start(
                ring0[:, 0:128].rearrange("p (k c) -> p k c", k=8),
                h0f[:].rearrange("(k p) c -> p k c", p=128))
            nc.sync.dma_start(
                H1T[:, 0:128].rearrange("p (k c) -> p k c", k=8),
                h1f[:].rearrange("(k p) c -> p k c", p=128))
            nc.sync.dma_start(h_own0[0][:], h0o[:])
            nc.sync.dma_start(h_own1[0][:], h1o[:])

            # ---- P1: gather + transpose x ----
            with (
                tc.tile_pool(name="gp", bufs=2) as gp,
                tc.tile_pool(name="gps", bufs=2, space="PSUM") as gpp,
            ):
                idx_sb = gp.tile([128, NT], i32, tag="idx", name="idx")
                nc.sync.dma_start(idx_sb[:], idx[:].rearrange("a b -> b a"))
                ident = gp.tile([128, 128], fp32, tag="ident", name="ident")
                make_identity(nc, ident[:])
                for i in range(NT):
                    xg = gp.tile([128, E], fp32, tag="xg", name=f"xg{i}", bufs=2)
                    nc.gpsimd.indirect_dma_start(
                        out=xg[:], out_offset=None, in_=emb[:],
                        in_offset=bass.IndirectOffsetOnAxis(ap=idx_sb[:, i:i + 1], axis=0))
                    for e in range(4):
                        pt = gpp.tile([128, 128], fp32, tag="pt", name=f"pt{i}_{e}")
                        nc.tensor.transpose(pt[:], xg[:, 128 * e:128 * (e + 1)], ident[:])
                        nc.vector.tensor_copy(
                            XT[:, (T * B) * e + 128 * i: (T * B) * e + 128 * (i + 1)], pt[:])

            # ---- P2a: bulk input-gate precompute for layer 0 ----
            gi_rz = st.tile([128, 32 * T], bf16, tag="gi_rz", name="gi_rz")
            gi_n = st.tile([128, 16 * T], bf16, tag="gi_n", name="gi_n")
            gi_rz3 = gi_rz[:].rearrange("p (s c) -> p s c", c=32)
            gi_n3 = gi_n[:].rearrange("p (s c) -> p s c", c=16)
            with tc.tile_pool(name="gi_ps", bufs=2, space="PSUM") as gip:
                for mg in range(3):
                    for c in range(0, T * B, 512):
                        cw = min(512, T * B - c)
                        pg = gip.tile([128, cw], fp32, tag="pg", name=f"pg{mg}_{c}", bufs=2)
                        for e in range(4):
                            nc.tensor.matmul(
                                pg[:], wih0_sb[:, 3 * HS * e + 128 * mg:3 * HS * e + 128 * (mg + 1)],
                                XT[:, (T * B) * e + c:(T * B) * e + c + cw],
                                start=(e == 0), stop=(not USE_BIAS and e == 3))
                        if USE_BIAS:
                            nc.tensor.matmul(
                                pg[:], ba0_sb[0:1, 128 * mg:128 * (mg + 1)],
                                ones_sb[0:1, 0:cw], start=False, stop=True)
                        pg3 = pg[:].rearrange("p (s c) -> p s c", c=16)
                        nsteps = cw // 16
                        s0 = c // 16
                        if mg == 0:
                            nc.vector.tensor_copy(gi_rz3[:, s0:s0 + nsteps, 0:16], pg3)
                        elif mg == 1:
                            nc.vector.tensor_copy(gi_rz3[:, s0:s0 + nsteps, 16:32], pg3)
                        else:
                            nc.vector.tensor_copy(gi_n3[:, s0:s0 + nsteps, :], pg3)

            # ---- P2: recurrence supersteps ----
            def l0_groups(ps, rhs_h_fn):
                # r, z, h_n psum groups (x-parts live in gi_rz / gi_n)
                for gi, (c0, gc) in enumerate([(0, 0), (16, HS), (32, 2 * HS)]):
                    nbias = USE_BIAS and gi == 2
                    for k in range(8):
                        nc.tensor.matmul(ps[:, c0:c0 + 16],
                                         whh0_sb[:, 3 * HS * k + gc:3 * HS * k + gc + 128],
                                         rhs_h_fn(k),
                                         start=(k == 0), stop=(k == 7 and not nbias))
                    if nbias:
                        nc.tensor.matmul(ps[:, c0:c0 + 16], bb0_sb[0:1, 0:HS],
                                         ones_sb[0:1, 0:B], start=False, stop=True)

            def l1_groups(ps, rhs_h_fn, rhs_x_fn):
                for gi, (c0, gc) in enumerate([(0, 0), (16, HS), (32, 2 * HS)]):
                    # r, z: h+x(+bias); i_n (c0=32): x only (+ bias)
                    mms = []
                    if gi < 2:
                        for k in range(8):
                            mms.append((whh1_sb[:, 3 * HS * k + gc:3 * HS * k + gc + 128],
                                        rhs_h_fn(k)))
                    for k in range(8):
                        mms.append((wih1_sb[:, 3 * HS * k + gc:3 * HS * k + gc + 128],
                                    rhs_x_fn(k)))
                    if USE_BIAS:
                        mms.append((ba1_sb[0:1, gc:gc + 128], ones_sb[0:1, 0:B]))
                    for mi, (lt, rr) in enumerate(mms):
                        nc.tensor.matmul(ps[:, c0:c0 + 16], lt, rr,
                                         start=(mi == 0), stop=(mi == len(mms) - 1))
                mms = [(whh1_sb[:, 3 * HS * k + 2 * HS:3 * HS * k + 2 * HS + 128], rhs_h_fn(k))
                       for k in range(8)]
                if USE_BIAS:
                    mms.append((bb1_sb[0:1, 0:HS], ones_sb[0:1, 0:B]))
                for mi, (lt, rr) in enumerate(mms):
                    nc.tensor.matmul(ps[:, 48:64], lt, rr,
                                     start=(mi == 0), stop=(mi == len(mms) - 1))

            def gates(rz_pre, hn_ap, in_ap, hprev, hnew, cc_dst, tag, t):
                rz = st.tile([128, 32], fp32, tag=f"rz{tag}", name=f"rz{tag}_{t}", bufs=2)
                nc.scalar.activation(rz[:], rz_pre, mybir.ActivationFunctionType.Sigmoid)
                t1 = st.tile([128, B], fp32, tag=f"t1{tag}", name=f"t1{tag}_{t}", bufs=2)
                nc.vector.tensor_tensor(out=t1[:], in0=hn_ap, in1=rz[:, 0:16],
                                        op=mybir.AluOpType.mult)
                nc.vector.tensor_tensor(out=t1[:], in0=t1[:], in1=in_ap,
                                        op=mybir.AluOpType.add)
                nsb = st.tile([128, B], fp32, tag=f"n{tag}", name=f"n{tag}_{t}", bufs=2)
                nc.scalar.activation(nsb[:], t1[:], mybir.ActivationFunctionType.Tanh)
                a = st.tile([128, B], fp32, tag=f"a{tag}", name=f"a{tag}_{t}", bufs=2)
                nc.vector.tensor_tensor(out=a[:], in0=hprev[:], in1=nsb[:],
                                        op=mybir.AluOpType.subtract)
                nc.vector.tensor_tensor(out=a[:], in0=a[:], in1=rz[:, 16:32],
                                        op=mybir.AluOpType.mult)
                nc.vector.tensor_tensor(out=hnew[:], in0=a[:], in1=nsb[:],
                                        op=mybir.AluOpType.add)
                nc.vector.tensor_copy(cc_dst, hnew[:])

            psp_cm = tc.tile_pool(name="ps", bufs=2, space="PSUM")
            psp = psp_cm.__enter__()
            mp_cm = tc.tile_pool(name="mp", bufs=1)
            mp = mp_cm.__enter__()
            mpp_cm = tc.tile_pool(name="mps", bufs=1, space="PSUM")
            mpp = mpp_cm.__enter__()

            GC = 128
            NG = (T * B) // GC
            H1T3 = H1T[:].rearrange("p (t c) -> p t c", c=128)

            cc_srcs = {t: st.tile([128, 32], bf16, tag="cc_src", name=f"cc_src{t}", bufs=4)
                       for t in range(1, T + 5)}
            for tt in (1, 2):
                nc.vector.memset(cc_srcs[tt][:, 16:32], 0.0)
            for tt in (T + 1, T + 2):
                nc.vector.memset(cc_srcs[tt][:, 0:16], 0.0)

            p5_state = {}

            def p5_items(g):
                # returns list of emission closures for tb-row tile g (128 rows)
                items = []

                def rt_item(m, g=g):
                    if ("RT", g) not in p5_state:
                        p5_state[("RT", g)] = mp.tile([128, 8 * GC], bf16, tag="RTg",
                                                      name=f"RTg{g}", bufs=2)
                    RTg = p5_state[("RT", g)]
                    pr = mpp.tile([128, GC], fp32, tag="pr", name=f"pr{g}_{m}", bufs=1)
                    for k in range(8):
                        nc.tensor.matmul(
                            pr[:], w1t_sb[:, H * k + 128 * m:H * k + 128 * (m + 1)],
                            H1T3[:, 8 * g + 1:8 * (g + 1) + 1, 16 * k:16 * (k + 1)],
                            start=(k == 0), stop=(k == 7))
                    nc.scalar.activation(
                        RTg[:, GC * m:GC * (m + 1)], pr[:],
                        mybir.ActivationFunctionType.Relu, bias=b1_sb[:, m:m + 1], scale=1.0)

                def out_item(m, g=g):
                    if ("OUT", g) not in p5_state:
                        p5_state[("OUT", g)] = mp.tile([128, 4 * GC], bf16, tag="OUTg",
                                                       name=f"OUTg{g}", bufs=2)
                    RTg = p5_state[("RT", g)]
                    OUTg = p5_state[("OUT", g)]
                    po = mpp.tile([128, GC], fp32, tag="po", name=f"po{g}_{m}", bufs=1)
                    for k in range(8):
                        nc.tensor.matmul(
                            po[:], w2t_sb[:, E * k + 128 * m:E * k + 128 * (m + 1)],
                            RTg[:, GC * k:GC * (k + 1)],
                            start=(k == 0), stop=(k == 7))
                    nc.scalar.activation(
                        OUTg[:, GC * m:GC * (m + 1)], po[:],
                        mybir.ActivationFunctionType.Identity, bias=b2_sb[:, m:m + 1], scale=1.0)

                def lg_item(nchs, g=g):
                    if ("L", g) not in p5_state:
                        p5_state[("L", g)] = mp.tile([128, VS], fp32, tag="lsb",
                                                     name=f"lsb{g}", bufs=2)
                    OUTg = p5_state[("OUT", g)]
                    lsb = p5_state[("L", g)]
                    pl = mpp.tile([128, 500], fp32, tag="pl", name=f"pl{g}_{nchs}", bufs=2)
                    for e in range(4):
                        nc.tensor.matmul(
                            pl[:], OUTg[:, GC * e:GC * e + 128],
                            embts_sb[:, VS * e + nchs:VS * e + nchs + 500],
                            start=(e == 0), stop=False)
                    nc.tensor.matmul(
                        pl[:], ones_sb[0:1, 0:128], bgen_sb[0:1, nchs:nchs + 500],
                        start=False, stop=True)
                    nc.vector.tensor_copy(lsb[:, nchs:nchs + 500], pl[:])

                def dma_item(g=g):
                    lsb = p5_state[("L", g)]
                    nc.gpsimd.dma_start(out[128 * g:128 * (g + 1), 0:VS // 2],
                                        lsb[:, 0:VS // 2])
                    nc.gpsimd.dma_start(out[128 * g:128 * (g + 1), VS // 2:VS],
                                        lsb[:, VS // 2:VS])

                for m in range(8):
                    items.append(lambda m=m: rt_item(m))
                for m in range(4):
                    items.append(lambda m=m: out_item(m))
                for nchs in range(0, VS, 500):
                    items.append(lambda n=nchs: lg_item(n))
                items.append(dma_item)
                return items

            p5_queue = []
            p5_next_g = 0

            def p5_drain(t, budget):
                nonlocal_ns = {}
                return budget

            for t in range(1, T + 3):
                cc_src = cc_srcs[t]
                if t <= T:
                    # layer 0: h0_t  (critical chain)
                    ps0 = psp.tile([128, 48], fp32, tag="ps0", name=f"ps0_{t}", bufs=2)
                    l0_groups(
                        ps0,
                        lambda k: ring0[:, 128 * ((t - 1) % 4) + 16 * k:128 * ((t - 1) % 4) + 16 * k + 16])
                    trz = st.tile([128, 32], fp32, tag="trz", name=f"trz{t}", bufs=2)
                    nc.vector.tensor_tensor(out=trz[:], in0=ps0[:, 0:32],
                                            in1=gi_rz[:, 32 * (t - 1):32 * t],
                                            op=mybir.AluOpType.add)
                    gates(trz[:], ps0[:, 32:48], gi_n[:, 16 * (t - 1):16 * t],
                          h_own0[(t - 1) % 2], h_own0[t % 2], cc_src[:, 0:16], "L0", t)

                ccin = dr.tile([128, 32], bf16, tag="ccin", name=f"ccin{t}", bufs=2)
                ccout = dr.tile([128 * N, 32], bf16, tag="ccout", name=f"ccout{t}",
                                bufs=2, addr_space="Shared")
                nc.sync.dma_start(ccin[:], cc_src[:])
                nc.gpsimd.collective_compute(
                    "AllGather", mybir.AluOpType.bypass,
                    replica_groups=[list(range(N))],
                    ins=[ccin[:]], outs=[ccout[:]])
                if t <= T:
                    nc.sync.dma_start(
                        ring0[:, 128 * (t % 4):128 * (t % 4) + 128].rearrange("p (k c) -> p k c", k=8),
                        ccout[:, 0:16].rearrange("(k p) c -> p k c", p=128))
                if t >= 3:
                    nc.scalar.dma_start(
                        H1T[:, 128 * (t - 2):128 * (t - 1)].rearrange("p (k c) -> p k c", k=8),
                        ccout[:, 16:32].rearrange("(k p) c -> p k c", p=128))

                if 2 <= t <= T + 1:
                    # layer 1: h1_{t-1} (rides AG t+1; PE does this during AG flight)
                    ps1 = psp.tile([128, 64], fp32, tag="ps1", name=f"ps1_{t}", bufs=2)
                    l1_groups(
                        ps1,
                        lambda k: H1T[:, 128 * (t - 2) + 16 * k:128 * (t - 2) + 16 * k + 16],
                        lambda k: ring0[:, 128 * ((t - 1) % 4) + 16 * k:128 * ((t - 1) % 4) + 16 * k + 16])
                    gates(ps1[:, 0:32], ps1[:, 48:64], ps1[:, 32:48],
                          h_own1[(t - 2) % 2], h_own1[(t - 1) % 2],
                          cc_srcs[t + 1][:, 16:32], "L1", t)

                # interleave vocab-projection work once its h1 block landed
                while p5_next_g < NG and t >= 8 * p5_next_g + 10:
                    p5_queue.extend(p5_items(p5_next_g))
                    p5_next_g += 1
                for _ in range(3):
                    if p5_queue:
                        p5_queue.pop(0)()
            while p5_next_g < NG:
                p5_queue.extend(p5_items(p5_next_g))
                p5_next_g += 1
            for it in p5_queue:
                it()
            p5_queue = []

            mpp_cm.__exit__(None, None, None)
            mp_cm.__exit__(None, None, None)
            psp_cm.__exit__(None, None, None)

    nc.finalize()
    return nc


def _prep_inputs(hidden, trg, embedding, w_ih0, w_hh0, b_ih0, b_hh0,
                 w_ih1, w_hh1, b_ih1, b_hh1, w1, b1, w2, b2, b_gen):
    bf = ml_dtypes.bfloat16
    T = STEPS
    f32 = np.float32
    hidden = np.asarray(hidden, f32)
    trg = np.asarray(trg)
    embedding = np.asarray(embedding, f32)
    in_maps = []
    # (t,b) index order
    idx_full = np.asarray(trg.T[:T], np.int32).reshape(-1)          # (T*B,)
    idx_tiles = idx_full.reshape(-1, 128).astype(np.int32)          # (NT,128)

    def gslice(wT, r):
        # wT (K, 3H) -> (K, 384) slice of each gate for core r
        cols = np.concatenate([np.arange(HS) + g * H + r * HS for g in range(3)])
        return np.ascontiguousarray(wT[:, cols])

    for r in range(N):
        sl = slice(r * HS, (r + 1) * HS)
        ba0_ = (b_ih0 + b_hh0).astype(f32)
        ba0v = np.concatenate([ba0_[0 * H + r * HS:0 * H + (r + 1) * HS],
                               ba0_[1 * H + r * HS:1 * H + (r + 1) * HS],
                               np.asarray(b_ih0, f32)[2 * H + r * HS:2 * H + (r + 1) * HS]])
        bb0v = np.asarray(b_hh0, f32)[2 * H + r * HS:2 * H + (r + 1) * HS]
        ba1_ = (b_ih1 + b_hh1).astype(f32)
        ba1v = np.concatenate([ba1_[0 * H + r * HS:0 * H + (r + 1) * HS],
                               ba1_[1 * H + r * HS:1 * H + (r + 1) * HS],
                               np.asarray(b_ih1, f32)[2 * H + r * HS:2 * H + (r + 1) * HS]])
        bb1v = np.asarray(b_hh1, f32)[2 * H + r * HS:2 * H + (r + 1) * HS]
        in_maps.append({
            "emb": embedding,
            "idx": idx_tiles,
            "h0f": hidden[0].T.astype(bf),
            "h1f": hidden[1].T.astype(bf),
            "h0o": np.ascontiguousarray(hidden[0].T[sl]).astype(f32),
            "h1o": np.ascontiguousarray(hidden[1].T[sl]).astype(f32),
            "wih0": gslice(np.asarray(w_ih0, f32).T, r).astype(bf),
            "whh0": gslice(np.asarray(w_hh0, f32).T, r).astype(bf),
            "wih1": gslice(np.asarray(w_ih1, f32).T, r).astype(bf),
            "whh1": gslice(np.asarray(w_hh1, f32).T, r).astype(bf),
            "ba0": ba0v.reshape(1, -1).astype(bf),
            "bb0": bb0v.reshape(1, -1).astype(bf),
            "ba1": ba1v.reshape(1, -1).astype(bf),
            "bb1": bb1v.reshape(1, -1).astype(bf),
            "w1t": np.asarray(w1, f32).T.astype(bf),
            "b1c": np.asarray(b1, f32).reshape(8, 128).T.astype(f32),
            "w2t": np.asarray(w2, f32).T.astype(bf),
            "b2c": np.asarray(b2, f32).reshape(4, 128).T.astype(f32),
            "embts": np.ascontiguousarray(embedding.T[:, r * VS:(r + 1) * VS]).astype(bf),
            "bgen": np.asarray(b_gen, f32)[r * VS:(r + 1) * VS].reshape(1, -1).astype(bf),
        })
    return in_maps


def kernel(**inputs):
    from concourse.bass_utils import run_bass_kernel_spmd
    zb = not any(np.asarray(inputs[k]).any() for k in
                 ("b_ih0", "b_hh0", "b_ih1", "b_hh1", "b1", "b2", "b_gen"))
    key = ("nc", zb)
    if key not in _cache:
        _cache[key] = _build(USE_BIAS=not zb)
    nc = _cache[key]
    in_maps = _prep_inputs(**inputs)
    res = run_bass_kernel_spmd(nc, in_maps, core_ids=list(range(N)))
    T = STEPS
    outf = np.empty((B, T, V), np.float32)
    for r in range(N):
        lr = res.results[r]["out"].reshape(T, B, VS)
        outf[:, :, r * VS:(r + 1) * VS] = lr.transpose(1, 0, 2)
    return outf
